# revision 26
# baseline (speedup 1.0000x reference)
"""KroneckerMessage GNN message passing on 8 TRN2 NeuronCores (v2).

Strategy (self-contained, hardcoded for the nn_KroneckerMessage problem):
- Node phase (as v1): shard nodes 8-way; each core computes its slice of
  h = relu(LN(node_feat @ W_node + b_node)); AllGather h [npad, 20] f32.
- Expansion: each core locally builds h_tab [npad, 128] bf16 where row n =
  h(n,:) tiled 5x (100 slots) + pad, for dma_gather (256B elem rows).
- Edge phase: edges sharded by dst range; per 128-dst window, slots split
  into (src < 32768 | src >= 32768) groups, each padded to 128 multiples
  (dma_gather indices are int16). Per group of B windows:
    * one dma_gather (transpose) pulls h_sT [128, S] bf16 (feature-major)
      per lo/hi table half; one SBUF-source dma_gather pulls h_dT tiled
      [128, S] from the windows' own h blocks.
    * 4 replication matmuls (R_c one-hot [20,100]) expand h_sT to
      rep-each-20 chunks [100, S]; DVE TT rep*hd_tiled -> kron^T chunks
      in SBUF bf16 directly (no PE transposes, no PSUM copy).
    * per 128-edge subtile: 4 accumulating bf16 matmuls vs W_kron chunks;
      LN via DVE bn_stats/bn_aggr + batched ACT Rsqrt; fused
      normalize+relu on ACT -> y bf16; one-hot scatter matmul into the
      window accumulator in PSUM; flush once per window.
"""
import math

import ml_dtypes
import numpy as np

import concourse.bacc as bacc
import concourse.bass as bass
import concourse.tile as tile
from concourse import mybir
from concourse.bass_utils import run_bass_kernel_spmd
from concourse.library_config import mlp

N_CORES = 8
P = 128
D_NODE = 20
KRON = 400
KCH = 100
NCH = 4
LN_EPS = 1e-5
B_WIN = 4        # windows per gather group
CCH = 256        # edge columns per rep/kron chunk
GMAX = 512       # max idxs per dma_gather call (SWDGE ring limit)

# module-level knobs (test.py pokes these)
TRACE = False
TRACE_DIR = None
USE_SIM = False
NLO = 32768      # int16 gather split point (tests may shrink)

_BUILD_CACHE = {}


def _wrap16(a):
    """int16 idx array -> [128, n/16] wrapped layout (16 partitions x 8)."""
    a = np.asarray(a, np.int16)
    assert a.size % 16 == 0
    w = a.reshape(-1, 16).T  # [16, n/16]
    return np.tile(w, (8, 1))  # [128, n/16]


# --------------------------------------------------------------------------
# host-side prep
# --------------------------------------------------------------------------
def _prep(node_feat, W_node, b_node, g_node, beta_node, W_kron, b_kron,
          g_kron, beta_kron, src, dst):
    N, GF = node_feat.shape
    OUT = W_kron.shape[1]
    E = src.shape[0]
    assert GF % P == 0 and OUT == P and W_kron.shape[0] == KRON

    nodes_pc = int(math.ceil(N / (N_CORES * P))) * P
    npad = nodes_pc * N_CORES
    wpc = nodes_pc // P
    nlo = min(NLO, npad)

    src = np.asarray(src, np.int64)
    dst = np.asarray(dst, np.int64)

    # --- per (core, window) edge bucketing with lo/hi src split ----------
    core = dst // nodes_pc
    wloc = (dst % nodes_pc) // P
    is_hi = (src >= nlo).astype(np.int64)
    # order all edges by (core, window, hi, arbitrary)
    key = ((core * wpc + wloc) * 2 + is_hi)
    order = np.argsort(key, kind="stable")
    skey = key[order]
    counts = np.bincount(skey, minlength=N_CORES * wpc * 2)
    cnt = counts.reshape(N_CORES, wpc, 2)
    # padded group sizes, shared across cores (same program on all cores)
    SL = (np.ceil(cnt[:, :, 0].max(axis=0) / P) * P).astype(np.int64)  # [wpc]
    SH = (np.ceil(cnt[:, :, 1].max(axis=0) / P) * P).astype(np.int64)

    # groups of B_WIN windows; slots per group: [w0_lo..wB_lo, w0_hi..wB_hi]
    groups = []
    col = {"lo": 0, "hi": 0, "dst": 0, "sub": 0}
    slot_base = 0
    for g0 in range(0, wpc, B_WIN):
        wins = list(range(g0, min(g0 + B_WIN, wpc)))
        SLg = int(SL[wins].sum())
        SHg = int(SH[wins].sum())
        Sg = SLg + SHg
        info = dict(wins=wins, SLg=SLg, SHg=SHg, Sg=Sg,
                    col_lo=col["lo"], col_hi=col["hi"], col_dst=col["dst"],
                    sub0=col["sub"], slot0=slot_base)
        # per-window column ranges within the group's slot space
        lo_off, hi_off = 0, SLg
        rng = {}
        for w in wins:
            rng[w] = (lo_off, int(SL[w]), hi_off, int(SH[w]))
            lo_off += int(SL[w])
            hi_off += int(SH[w])
        info["rng"] = rng
        groups.append(info)
        col["lo"] += SLg // 16
        col["hi"] += SHg // 16
        col["dst"] += Sg // 16
        col["sub"] += Sg // P
        slot_base += Sg
    TS = slot_base                  # total slots per core
    CL, CH, CD = col["lo"], col["hi"], col["dst"]
    TSUB = col["sub"]

    # --- slot assembly per core -----------------------------------------
    starts = np.concatenate([[0], np.cumsum(counts)])
    # slot arrays (per core)
    idx_lo = np.zeros((N_CORES, 128, max(CL, 1)), np.int16)
    idx_hi = np.zeros((N_CORES, 128, max(CH, 1)), np.int16)
    idx_dst = np.zeros((N_CORES, 128, max(CD, 1)), np.int16)
    dloc = np.full((N_CORES, 128, max(TSUB, 1)), -1.0, np.float32)

    src_s = src[order]
    dst_s = dst[order]
    for c in range(N_CORES):
        for gi in groups:
            Sg = gi["Sg"]
            slo = np.zeros(gi["SLg"], np.int16)
            shi = np.zeros(gi["SHg"], np.int16)
            sdst = np.zeros(Sg, np.int16)
            sdl = np.full(Sg, -1.0, np.float32)
            for wi, w in enumerate(gi["wins"]):
                b_lo = starts[(c * wpc + w) * 2]
                n_lo = counts[(c * wpc + w) * 2]
                b_hi = starts[(c * wpc + w) * 2 + 1]
                n_hi = counts[(c * wpc + w) * 2 + 1]
                lo0, SLw, hi0, SHw = gi["rng"][w]
                hir = hi0 - gi["SLg"]
                slo[lo0:lo0 + n_lo] = src_s[b_lo:b_lo + n_lo].astype(np.int16)
                shi[hir:hir + n_hi] = (src_s[b_hi:b_hi + n_hi]
                                       - nlo).astype(np.int16)
                dl_lo = (dst_s[b_lo:b_lo + n_lo] % P).astype(np.float32)
                dl_hi = (dst_s[b_hi:b_hi + n_hi] % P).astype(np.float32)
                sdl[lo0:lo0 + n_lo] = dl_lo
                sdl[hi0:hi0 + n_hi] = dl_hi
                sdst[lo0:lo0 + n_lo] = (wi * P + dl_lo).astype(np.int16)
                sdst[hi0:hi0 + n_hi] = (wi * P + dl_hi).astype(np.int16)
            if gi["SLg"]:
                idx_lo[c, :, gi["col_lo"]:gi["col_lo"] + gi["SLg"] // 16] = \
                    _wrap16(slo)
            if gi["SHg"]:
                idx_hi[c, :, gi["col_hi"]:gi["col_hi"] + gi["SHg"] // 16] = \
                    _wrap16(shi)
            idx_dst[c, :, gi["col_dst"]:gi["col_dst"] + Sg // 16] = \
                _wrap16(sdst)
            dloc[c, :, gi["sub0"]:gi["sub0"] + Sg // P] = \
                sdl.reshape(-1, P).T

    nf_pad = np.zeros((npad, GF), np.float32)
    nf_pad[:N] = np.asarray(node_feat, np.float32)

    # W_node -> [P, GF//P, D_NODE]
    wn = np.asarray(W_node, np.float32).reshape(GF // P, P, D_NODE)
    wn = np.ascontiguousarray(wn.transpose(1, 0, 2))

    # W_kron chunks: [KCH, NCH, OUT] bf16 (chunk c covers rows c*100+p)
    wk = np.asarray(W_kron, np.float32).reshape(NCH, KCH, OUT)
    wk = np.ascontiguousarray(wk.transpose(1, 0, 2)).astype(ml_dtypes.bfloat16)

    # replication one-hots R[k, c, p] = (k == c*5 + p//20)
    R = np.zeros((D_NODE, NCH, KCH), np.float32)
    for c in range(NCH):
        for p_ in range(KCH):
            R[c * 5 + p_ // D_NODE, c, p_] = 1.0
    R = R.astype(ml_dtypes.bfloat16)

    flags = dict(
        has_bn=bool(np.any(np.asarray(b_node) != 0)),
        has_gn=bool(np.any(np.asarray(g_node) != 1)),
        has_betan=bool(np.any(np.asarray(beta_node) != 0)),
        has_bk=bool(np.any(np.asarray(b_kron) != 0)),
        has_gk=bool(np.any(np.asarray(g_kron) != 1)),
        has_betak=bool(np.any(np.asarray(beta_kron) != 0)),
    )

    cfg = dict(N=N, GF=GF, OUT=OUT, E=E, nodes_pc=nodes_pc, npad=npad,
               wpc=wpc, nlo=nlo, CL=CL, CH=CH, CD=CD, TSUB=TSUB, TS=TS,
               SL=tuple(int(x) for x in SL), SH=tuple(int(x) for x in SH),
               **flags)
    cfg["groups"] = tuple(
        (tuple(gi["wins"]), gi["SLg"], gi["SHg"], gi["col_lo"], gi["col_hi"],
         gi["col_dst"], gi["sub0"],
         tuple((w,) + gi["rng"][w] for w in gi["wins"])) for gi in groups)

    in_maps = []
    for c in range(N_CORES):
        m = dict(
            nf=nf_pad[c * nodes_pc:(c + 1) * nodes_pc],
            wn=wn,
            wk=wk,
            Rtab=R,
            idx_lo=idx_lo[c],
            idx_hi=idx_hi[c],
            idx_dst=idx_dst[c],
            dloc=dloc[c],
            b_node=np.asarray(b_node, np.float32),
            g_node=np.asarray(g_node, np.float32),
            beta_node=np.asarray(beta_node, np.float32),
            bk=np.asarray(b_kron, np.float32).astype(ml_dtypes.bfloat16),
            g_kron=np.asarray(g_kron, np.float32).astype(ml_dtypes.bfloat16),
            beta_kron=np.asarray(beta_kron, np.float32).astype(
                ml_dtypes.bfloat16),
            iota_row=np.arange(P, dtype=np.float32),
        )
        in_maps.append(m)
    return cfg, in_maps


# --------------------------------------------------------------------------
# device program
# --------------------------------------------------------------------------
def _build(cfg):
    GF, OUT = cfg["GF"], cfg["OUT"]
    nodes_pc, npad, wpc = cfg["nodes_pc"], cfg["npad"], cfg["wpc"]
    nlo = cfg["nlo"]
    FCH = GF // P
    f32, bf16, i16 = mybir.dt.float32, mybir.dt.bfloat16, mybir.dt.int16
    TSUB = cfg["TSUB"]

    nc = bacc.Bacc(num_devices=N_CORES)
    nf = nc.dram_tensor("nf", [nodes_pc, GF], f32, kind="ExternalInput")
    wn = nc.dram_tensor("wn", [P, FCH, D_NODE], f32, kind="ExternalInput")
    wk = nc.dram_tensor("wk", [KCH, NCH, OUT], bf16, kind="ExternalInput")
    Rtab = nc.dram_tensor("Rtab", [D_NODE, NCH, KCH], bf16,
                          kind="ExternalInput")
    idx_lo = nc.dram_tensor("idx_lo", [P, max(cfg["CL"], 1)], i16,
                            kind="ExternalInput")
    idx_hi = nc.dram_tensor("idx_hi", [P, max(cfg["CH"], 1)], i16,
                            kind="ExternalInput")
    idx_dst = nc.dram_tensor("idx_dst", [P, max(cfg["CD"], 1)], i16,
                             kind="ExternalInput")
    dloc = nc.dram_tensor("dloc", [P, max(TSUB, 1)], f32,
                          kind="ExternalInput")
    b_node = nc.dram_tensor("b_node", [D_NODE], f32, kind="ExternalInput")
    g_node = nc.dram_tensor("g_node", [D_NODE], f32, kind="ExternalInput")
    beta_node = nc.dram_tensor("beta_node", [D_NODE], f32,
                               kind="ExternalInput")
    bk = nc.dram_tensor("bk", [OUT], bf16, kind="ExternalInput")
    g_kron = nc.dram_tensor("g_kron", [OUT], bf16, kind="ExternalInput")
    beta_kron = nc.dram_tensor("beta_kron", [OUT], bf16,
                               kind="ExternalInput")
    iota_row = nc.dram_tensor("iota_row", [P], f32, kind="ExternalInput")

    out_part = nc.dram_tensor("out_part", [nodes_pc, OUT], f32,
                              kind="ExternalOutput")
    h_part = nc.dram_tensor("h_part", [nodes_pc, D_NODE], f32)
    h_full = nc.dram_tensor("h_full", [npad, D_NODE], f32,
                            addr_space="Shared")
    h_tab = nc.dram_tensor("h_tab", [npad, P], bf16)
    h_tab_loc = nc.dram_tensor("h_tab_loc", [nodes_pc, P], bf16)

    ntiles = nodes_pc // P

    # ---------------- phase 1: h = relu(LN(nf @ W_node + b)) --------------
    from concourse.masks import make_identity
    with tile.TileContext(nc) as tc:
        with (
            tc.tile_pool(name="hconst", bufs=1) as hconst,
            tc.tile_pool(name="hsb", bufs=3) as hsb,
            tc.tile_pool(name="hps", bufs=2, space="PSUM") as hps,
            tc.tile_pool(name="hsmall", bufs=4) as hsmall,
        ):
            ident_f32 = hconst.tile([P, P], f32)
            make_identity(nc, ident_f32[:])
            wn_sb = hconst.tile([P, FCH, D_NODE], f32)
            nc.gpsimd.dma_start(out=wn_sb[:], in_=wn[:])
            eps_t = hconst.tile([P, 1], f32)
            nc.vector.memset(eps_t[:], LN_EPS)
            if cfg["has_bn"]:
                bn_b = hconst.tile([P, D_NODE], f32)
                nc.gpsimd.dma_start(
                    out=bn_b[:],
                    in_=bass.AP(tensor=b_node, offset=0,
                                ap=[[0, P], [1, D_NODE]]))
            if cfg["has_gn"]:
                gn_b = hconst.tile([P, D_NODE], f32)
                nc.gpsimd.dma_start(
                    out=gn_b[:],
                    in_=bass.AP(tensor=g_node, offset=0,
                                ap=[[0, P], [1, D_NODE]]))
            if cfg["has_betan"]:
                betan_b = hconst.tile([P, D_NODE], f32)
                nc.gpsimd.dma_start(
                    out=betan_b[:],
                    in_=bass.AP(tensor=beta_node, offset=0,
                                ap=[[0, P], [1, D_NODE]]))

            h_stage = hconst.tile([P, ntiles, D_NODE], f32)

            for t in range(ntiles):
                nf_t = hsb.tile([P, GF], f32, tag="nf_t")
                nc.gpsimd.dma_start(out=nf_t[:], in_=nf[t * P:(t + 1) * P, :])
                nfT_ps = hps.tile([P, FCH, P], f32, tag="nfT_ps")
                for c in range(FCH):
                    nc.tensor.transpose(out=nfT_ps[:, c, :],
                                        in_=nf_t[:, c * P:(c + 1) * P],
                                        identity=ident_f32[:])
                nfT = hsb.tile([P, FCH, P], f32, tag="nfT")
                nc.vector.tensor_copy(out=nfT[:], in_=nfT_ps[:])
                z_ps = hps.tile([P, D_NODE], f32, tag="z_ps")
                for c in range(FCH):
                    nc.tensor.matmul(out=z_ps[:], lhsT=nfT[:, c, :],
                                     rhs=wn_sb[:, c, :], start=(c == 0),
                                     stop=(c == FCH - 1))
                if cfg["has_bn"]:
                    z_sb = hsb.tile([P, D_NODE], f32, tag="z_sb")
                    nc.vector.tensor_add(out=z_sb[:], in0=z_ps[:], in1=bn_b[:])
                    z_in = z_sb
                else:
                    z_in = z_ps
                stats = hsmall.tile([P, 6], f32, tag="stats")
                nc.vector.bn_stats(out=stats[:], in_=z_in[:])
                mv = hsmall.tile([P, 2], f32, tag="mv")
                nc.vector.bn_aggr(out=mv[:], in_=stats[:])
                sd = hsmall.tile([P, 1], f32, tag="sd")
                nc.scalar.activation(out=sd[:], in_=mv[:, 1:2],
                                     func=mybir.ActivationFunctionType.Sqrt,
                                     bias=eps_t[:], scale=1.0)
                rstd = hsmall.tile([P, 1], f32, tag="rstd")
                nc.vector.reciprocal(out=rstd[:], in_=sd[:])
                nmr = hsmall.tile([P, 1], f32, tag="nmr")
                nc.vector.tensor_scalar(out=nmr[:], in0=mv[:, 0:1],
                                        scalar1=rstd[:, 0:1], scalar2=-1.0,
                                        op0=mybir.AluOpType.mult,
                                        op1=mybir.AluOpType.mult)
                simple = not (cfg["has_gn"] or cfg["has_betan"])
                func = (mybir.ActivationFunctionType.Relu if simple
                        else mybir.ActivationFunctionType.Identity)
                nc.scalar.activation(out=h_stage[:, t, :], in_=z_in[:],
                                     func=func, bias=nmr[:],
                                     scale=rstd[:, 0:1])
                if not simple:
                    if cfg["has_gn"]:
                        nc.vector.tensor_mul(out=h_stage[:, t, :],
                                             in0=h_stage[:, t, :],
                                             in1=gn_b[:])
                    if cfg["has_betan"]:
                        nc.vector.tensor_add(out=h_stage[:, t, :],
                                             in0=h_stage[:, t, :],
                                             in1=betan_b[:])
                    nc.vector.tensor_scalar_max(out=h_stage[:, t, :],
                                                in0=h_stage[:, t, :],
                                                scalar1=0.0)
            nc.sync.dma_start(
                out=h_part.rearrange("(t p) d -> p t d", p=P),
                in_=h_stage[:])

    # ---------------- collective: AllGather h ----------------------------
    with (
        nc.Block() as block,
        nc.semaphore("cc_sem") as cc_sem,
    ):
        @block.gpsimd
        def _(gpsimd):
            gpsimd.collective_compute(
                "AllGather",
                mybir.AluOpType.bypass,
                replica_groups=[list(range(N_CORES))],
                ins=[h_part[:]],
                outs=[h_full[:]],
            ).then_inc(cc_sem)
            gpsimd.wait_ge(cc_sem, 1)
            gpsimd.sem_clear(cc_sem)
            gpsimd.load_library(mlp)

    # ---------------- expansion: h_full -> h_tab (tiled 5x bf16) ----------
    XT = 8  # node tiles per expansion iteration
    with tile.TileContext(nc) as tc:
        with (
            tc.tile_pool(name="xin", bufs=3) as xin,
            tc.tile_pool(name="xout", bufs=3) as xout,
        ):
            nxi = npad // (XT * P)
            rem = npad - nxi * XT * P
            engs = [nc.vector, nc.scalar, nc.gpsimd]
            it = 0
            # local dst table from the core's own h_part
            nloc = nodes_pc // (XT * P)
            remloc = nodes_pc - nloc * XT * P
            for i in range(nloc + (1 if remloc else 0)):
                tcount = XT if i < nloc else remloc // P
                r0 = i * XT * P
                in_t = xin.tile([P, tcount, D_NODE], f32, tag="xinl")
                nc.sync.dma_start(
                    out=in_t[:],
                    in_=h_part[r0:r0 + tcount * P].rearrange(
                        "(t p) d -> p t d", p=P))
                out_t = xout.tile([P, tcount, P], bf16, tag="xoutl")
                nc.vector.memset(out_t[:, :, KCH:P], 0.0)
                nc.vector.tensor_copy(
                    out=out_t[:, :, 0:KCH].rearrange(
                        "p t (f d) -> p t f d", f=5),
                    in_=in_t[:, :, None, :].to_broadcast(
                        [P, tcount, 5, D_NODE]))
                nc.sync.dma_start(
                    out=h_tab_loc[r0:r0 + tcount * P, :].rearrange(
                        "(t p) d -> p t d", p=P),
                    in_=out_t[:])
            for i in range(nxi + (1 if rem else 0)):
                tcount = XT if i < nxi else rem // P
                r0 = i * XT * P
                in_t = xin.tile([P, tcount, D_NODE], f32, tag="xin")
                nc.sync.dma_start(
                    out=in_t[:],
                    in_=h_full[r0:r0 + tcount * P].rearrange(
                        "(t p) d -> p t d", p=P))
                out_t = xout.tile([P, tcount, P], bf16, tag="xout")
                eng = engs[it % 3]
                it += 1
                eng2 = engs[it % 3]
                it += 1
                nc.vector.memset(out_t[:, :, KCH:P], 0.0)
                bcast = in_t[:, :, None, :].to_broadcast(
                    [P, tcount, 5, D_NODE])
                out_v = out_t[:, :, 0:KCH].rearrange(
                    "p t (f d) -> p t f d", f=5)
                if eng2 is nc.scalar:
                    nc.scalar.activation(
                        out=out_v, in_=bcast,
                        func=mybir.ActivationFunctionType.Identity, scale=1.0)
                else:
                    eng2.tensor_copy(out=out_v, in_=bcast)
                nc.sync.dma_start(
                    out=h_tab[r0:r0 + tcount * P, :].rearrange(
                        "(t p) d -> p t d", p=P),
                    in_=out_t[:])

    # ---------------- phase 2: edges --------------------------------------
    simple_k = not (cfg["has_gk"] or cfg["has_betak"])
    groups = cfg["groups"]
    with tile.TileContext(nc) as tc:
        with (
            tc.tile_pool(name="econst", bufs=1) as econst,
            tc.tile_pool(name="eg", bufs=2) as eg,
            tc.tile_pool(name="ek", bufs=2) as ek,
            tc.tile_pool(name="esb", bufs=3) as esb,
            tc.tile_pool(name="eps_r", bufs=2, space="PSUM") as epsR,
            tc.tile_pool(name="eps_z", bufs=2, space="PSUM") as epsZ,
            tc.tile_pool(name="eps_a", bufs=2, space="PSUM") as epsA,
            tc.tile_pool(name="esmall", bufs=6) as esmall,
        ):
            iota_f = econst.tile([P, P], f32)
            nc.gpsimd.dma_start(
                out=iota_f[:], in_=bass.AP(tensor=iota_row, offset=0,
                                           ap=[[0, P], [1, P]]))
            eps_t2 = econst.tile([P, 1], f32)
            nc.vector.memset(eps_t2[:], LN_EPS)
            wk_sb = econst.tile([KCH, NCH, OUT], bf16)
            nc.gpsimd.dma_start(out=wk_sb[:], in_=wk[:])
            R_sb = econst.tile([D_NODE, NCH, KCH], bf16)
            nc.gpsimd.dma_start(out=R_sb[:], in_=Rtab[:])
            if cfg["has_bk"]:
                ones_row = econst.tile([1, P], bf16)
                nc.vector.memset(ones_row[:], 1.0)
                bk_sb = econst.tile([1, OUT], bf16)
                nc.gpsimd.dma_start(out=bk_sb[:], in_=bk[None, :])
            if cfg["has_gk"]:
                gk_b = econst.tile([P, OUT], bf16)
                nc.gpsimd.dma_start(
                    out=gk_b[:], in_=bass.AP(tensor=g_kron, offset=0,
                                             ap=[[0, P], [1, OUT]]))
            if cfg["has_betak"]:
                betak_b = econst.tile([P, OUT], bf16)
                nc.gpsimd.dma_start(
                    out=betak_b[:],
                    in_=bass.AP(tensor=beta_kron, offset=0,
                                ap=[[0, P], [1, OUT]]))

            for (wins, SLg, SHg, col_lo, col_hi, col_dst, sub0,
                 rng) in groups:
                Sg = SLg + SHg
                BW = len(wins)
                w0 = wins[0]
                # window h blocks [128, BW, 128] bf16 (rank stripes)
                blocks = eg.tile([P, BW, P], bf16, tag="blocks")
                nc.sync.dma_start(
                    out=blocks[:],
                    in_=h_tab_loc[w0 * P:(w0 + BW) * P, :].rearrange(
                        "(b p) d -> p b d", p=P))
                # index tiles
                if SLg:
                    ilo = eg.tile([P, SLg // 16], i16, tag="ilo")
                    nc.sync.dma_start(
                        out=ilo[:],
                        in_=idx_lo[:, col_lo:col_lo + SLg // 16])
                if SHg:
                    ihi = eg.tile([P, SHg // 16], i16, tag="ihi")
                    nc.sync.dma_start(
                        out=ihi[:],
                        in_=idx_hi[:, col_hi:col_hi + SHg // 16])
                idst = eg.tile([P, Sg // 16], i16, tag="idst")
                nc.sync.dma_start(
                    out=idst[:], in_=idx_dst[:, col_dst:col_dst + Sg // 16])
                dl_t = eg.tile([P, Sg // P], f32, tag="dl")
                nc.sync.dma_start(out=dl_t[:],
                                  in_=dloc[:, sub0:sub0 + Sg // P])

                # gathers (split into <=GMAX-idx calls: SWDGE ring capacity)
                hs_lo = hs_hi = None
                if SLg:
                    hs_lo = eg.tile([P, SLg], bf16, tag="hs_lo")
                    for off in range(0, SLg, GMAX):
                        n = min(GMAX, SLg - off)
                        nc.gpsimd.dma_gather(
                            out_ap=hs_lo[:, None, off:off + n],
                            in_ap=h_tab[0:nlo, :],
                            idxs_ap=ilo[:, off // 16:(off + n) // 16],
                            num_idxs=n, num_idxs_reg=n,
                            elem_size=P, elem_step=P, transpose=True)
                if SHg:
                    hs_hi = eg.tile([P, SHg], bf16, tag="hs_hi")
                    for off in range(0, SHg, GMAX):
                        n = min(GMAX, SHg - off)
                        nc.gpsimd.dma_gather(
                            out_ap=hs_hi[:, None, off:off + n],
                            in_ap=h_tab[nlo:npad, :],
                            idxs_ap=ihi[:, off // 16:(off + n) // 16],
                            num_idxs=n, num_idxs_reg=n,
                            elem_size=P, elem_step=P, transpose=True)
                hd = eg.tile([P, Sg], bf16, tag="hd")
                for off in range(0, Sg, GMAX):
                    n = min(GMAX, Sg - off)
                    nc.gpsimd.dma_gather(
                        out_ap=hd[:, None, off:off + n],
                        in_ap=blocks[:].rearrange("p b d -> p (b d)"),
                        idxs_ap=idst[:, off // 16:(off + n) // 16],
                        num_idxs=n, num_idxs_reg=n,
                        elem_size=P, transpose=True,
                        sbuf_tokens_per_rank=P,
                        sbuf_free_dim_per_rank=2 * P,
                    )

                for (w, lo0, SLw, hi0, SHw) in rng:
                    nsub = (SLw + SHw) // P
                    if nsub == 0:
                        zero_sb = esb.tile([P, OUT], f32, tag="out_sb")
                        nc.vector.memset(zero_sb[:], 0.0)
                        nc.sync.dma_start(
                            out=out_part[w * P:(w + 1) * P, :],
                            in_=zero_sb[:])
                        continue
                    acc_ps = epsA.tile([P, OUT], f32, tag="acc")
                    si = 0
                    # chunked column ranges for this window
                    for (base, width, hsrc) in (
                            (lo0, SLw, "lo"), (hi0, SHw, "hi")):
                        hs_t = hs_lo if hsrc == "lo" else hs_hi
                        hs_off = 0 if hsrc == "lo" else SLg
                        c0 = base
                        while c0 < base + width:
                            wdt = min(CCH, base + width - c0)
                            hc0 = c0 - hs_off
                            rep = epsR.tile([KCH, NCH, CCH], f32, tag="rep")
                            for c in range(NCH):
                                nc.tensor.matmul(
                                    out=rep[:, c, 0:wdt],
                                    lhsT=R_sb[:, c, :],
                                    rhs=hs_t[0:D_NODE, hc0:hc0 + wdt],
                                    start=True, stop=True)
                            kron = ek.tile([KCH, NCH, CCH], bf16, tag="kron")
                            for c in range(NCH):
                                nc.vector.tensor_tensor(
                                    out=kron[:, c, 0:wdt],
                                    in0=rep[:, c, 0:wdt],
                                    in1=hd[0:KCH, c0:c0 + wdt],
                                    op=mybir.AluOpType.mult)
                            nsub_c = wdt // P
                            mv = esmall.tile([P, nsub_c, 2], f32, tag="mv")
                            rstd = esmall.tile([P, nsub_c], f32, tag="rstd")
                            nmr = esmall.tile([P, nsub_c], f32, tag="nmr")
                            y_list = []
                            for s in range(nsub_c):
                                z_ps = epsZ.tile([P, OUT], f32, tag="z")
                                nmm = NCH + (1 if cfg["has_bk"] else 0)
                                for c in range(NCH):
                                    nc.tensor.matmul(
                                        out=z_ps[:],
                                        lhsT=kron[:, c, s * P:(s + 1) * P],
                                        rhs=wk_sb[:, c, :], start=(c == 0),
                                        stop=(c == nmm - 1))
                                if cfg["has_bk"]:
                                    nc.tensor.matmul(
                                        out=z_ps[:], lhsT=ones_row[:],
                                        rhs=bk_sb[:], start=False, stop=True,
                                        skip_group_check=True)
                                stats = esmall.tile([P, 6], f32, tag="st6")
                                nc.vector.bn_stats(out=stats[:], in_=z_ps[:])
                                nc.vector.bn_aggr(out=mv[:, s, :],
                                                  in_=stats[:])
                                y_list.append(z_ps)
                            # batched rsqrt / nmr over the chunk
                            sd = esmall.tile([P, nsub_c], f32, tag="sd")
                            nc.scalar.activation(
                                out=sd[:], in_=mv[:, :, 1],
                                func=mybir.ActivationFunctionType.Sqrt,
                                bias=eps_t2[:], scale=1.0)
                            nc.vector.reciprocal(out=rstd[:], in_=sd[:])
                            nc.vector.tensor_tensor(
                                out=nmr[:], in0=mv[:, :, 0], in1=rstd[:],
                                op=mybir.AluOpType.mult)
                            nc.vector.tensor_scalar_mul(
                                out=nmr[:], in0=nmr[:], scalar1=-1.0)
                            for s in range(nsub_c):
                                z_ps = y_list[s]
                                y_sb = esb.tile([P, OUT], bf16, tag="y")
                                func = (mybir.ActivationFunctionType.Relu
                                        if simple_k else
                                        mybir.ActivationFunctionType.Identity)
                                nc.scalar.activation(
                                    out=y_sb[:], in_=z_ps[:], func=func,
                                    bias=nmr[:, s:s + 1],
                                    scale=rstd[:, s:s + 1])
                                if not simple_k:
                                    if cfg["has_gk"]:
                                        nc.vector.tensor_mul(
                                            out=y_sb[:], in0=y_sb[:],
                                            in1=gk_b[:])
                                    if cfg["has_betak"]:
                                        nc.vector.tensor_add(
                                            out=y_sb[:], in0=y_sb[:],
                                            in1=betak_b[:])
                                    nc.vector.tensor_scalar_max(
                                        out=y_sb[:], in0=y_sb[:], scalar1=0.0)
                                gsub = sub0 + (c0 + s * P) // P
                                oh = esb.tile([P, P], bf16, tag="oh")
                                nc.vector.tensor_scalar(
                                    out=oh[:], in0=iota_f[:],
                                    scalar1=dl_t[:, gsub - sub0:gsub - sub0
                                                 + 1],
                                    scalar2=None,
                                    op0=mybir.AluOpType.is_equal)
                                sub_idx = si + (c0 - base) // P + s
                                nc.tensor.matmul(
                                    out=acc_ps[:], lhsT=oh[:], rhs=y_sb[:],
                                    start=(sub_idx == 0),
                                    stop=(sub_idx == nsub - 1))
                            c0 += wdt
                        si += width // P

                    out_sb = esb.tile([P, OUT], f32, tag="out_sb")
                    nc.vector.tensor_copy(out=out_sb[:], in_=acc_ps[:])
                    nc.sync.dma_start(out=out_part[w * P:(w + 1) * P, :],
                                      in_=out_sb[:])

    nc.compile()
    return nc


# --------------------------------------------------------------------------
# entry point
# --------------------------------------------------------------------------
def _install_trace_hook():
    import sys, types, ctypes, contextlib
    if "antenv.axon_hooks" in sys.modules:
        return
    lib = ctypes.CDLL("/opt/axon/libaxon_pjrt.so")
    lib.axon_start_nrt_profile.argtypes = [ctypes.POINTER(ctypes.c_int64),
                                           ctypes.c_size_t]
    lib.axon_start_nrt_profile.restype = ctypes.c_int64
    lib.axon_stop_nrt_profile.argtypes = [ctypes.c_char_p]
    lib.axon_stop_nrt_profile.restype = ctypes.c_int64

    @contextlib.contextmanager
    def _hook(output_dir, device_ids):
        import jax
        jax.devices()
        if device_ids:
            ids = (ctypes.c_int64 * len(device_ids))(*device_ids)
            rc = lib.axon_start_nrt_profile(ids, len(device_ids))
        else:
            rc = lib.axon_start_nrt_profile(None, 0)
        if rc != 0:
            raise RuntimeError(f"axon_start_nrt_profile rc={rc}")
        try:
            yield
        finally:
            n = lib.axon_stop_nrt_profile(str(output_dir).encode())
            print(f"profile: {n} file(s) -> {output_dir}")

    mod = types.ModuleType("antenv.axon_hooks")
    mod.get_axon_ntff_profile_hook = lambda: _hook
    sys.modules["antenv.axon_hooks"] = mod
    from concourse import bass_utils
    bass_utils.upload_artifacts = lambda tmpdir: "local://skipped"


def kernel(**inputs):
    cfg, in_maps = _prep(**inputs)
    key = (cfg["N"], cfg["GF"], cfg["OUT"], cfg["E"], cfg["TS"],
           cfg["groups"], cfg["has_bn"], cfg["has_gn"], cfg["has_betan"],
           cfg["has_bk"], cfg["has_gk"], cfg["has_betak"])
    if key not in _BUILD_CACHE:
        _BUILD_CACHE[key] = _build(cfg)
    nc = _BUILD_CACHE[key]

    if USE_SIM:
        from concourse import bass_interp
        sim = bass_interp.MultiCoreSim(nc, N_CORES)
        for c in range(N_CORES):
            for name, arr in in_maps[c].items():
                sim.cores[c].tensor(name)[:] = arr
        sim.simulate()
        parts = [np.array(sim.cores[c].tensor("out_part"))
                 for c in range(N_CORES)]
        exec_ns = None
    else:
        kw = {}
        if TRACE:
            _install_trace_hook()
            kw = dict(trace=True, tmpdir=TRACE_DIR)
        res = run_bass_kernel_spmd(nc, in_maps, list(range(N_CORES)), **kw)
        parts = [res.results[c]["out_part"] for c in range(N_CORES)]
        exec_ns = res.exec_time_ns
        kernel.last_exec_ns = exec_ns

    out = np.concatenate(parts, axis=0)[:cfg["N"]]
    return out.astype(np.float32)


kernel.last_exec_ns = None
